# revision 8
# baseline (speedup 1.0000x reference)
"""BLOOM attention layer on 8 Trainium2 NeuronCores.

Sharding: tensor-parallel over heads (4 heads/core) x data-parallel over batch
(B=2), mesh [DP=2, TP=4].  Core c handles batch b=c//4, heads 4*(c%4)..+3.

v3.1 design:
  - bf16 matmuls for QKV projection, scores and dense (fp8 fails the accuracy
    gate there); fp8e4 DoubleRow matmuls for the attention ctx and softmax-sum
    reductions: exp is written to fp8 E pairs [128, 2, 512] and each DR matmul
    contracts two 128-key tiles at one bf16-matmul's cost.  V is stored fp8.
    Scores are shifted by -2 (folded into alibi) so E stays inside e4m3 range;
    the shift cancels in the softmax ratio.
  - Per-head pipeline: QK projection of head h+1 (and the dense partials for
    the last head) are interleaved into attention of head h, which hides the
    scalar-engine exp stream and removes phase-boundary stalls.
  - Causal structure hardcoded: full key pairs plus diagonal pairs at widths
    (512,512) and (256,256) via nested partial-region PSUM accumulation; the
    only mask tiles are a shared 128x128 triangle and a 128x256 zero|triangle.
  - Softmax normalize: reciprocal_approx_fast + tensor_mul (the exact DVE
    reciprocal costs 3.4us/tile).  V bias is folded out on the host
    (softmax rows sum to 1 -> + bv passes through; bv @ W_dense joins
    b_dense).  Dense partials leave as bf16; host sums the 4 TP partials.
"""

import numpy as np
import ml_dtypes

bf16 = ml_dtypes.bfloat16
f8 = ml_dtypes.float8_e4m3fn

B, S, H, NH = 2, 2048, 2048, 16
HD = H // NH  # 128
INV_NORM = 1.0 / float(np.sqrt(HD))
NCORES = 8
TP = 4
HPC = NH // TP  # 4 heads per core
QSL = 512
KTL = 128
N_QS = S // QSL   # 4
N_KT = S // KTL   # 16
N_HT = H // 128   # 16 contraction tiles
NCI = 2 * HPC     # 8 q/k column tiles

ATTN_FP8 = True   # fp8e4 DoubleRow for ctx & softmax sums
EXP_SHIFT = 2.0   # scores shift folded into alibi (cancels in softmax)

_program_cache: dict = {}


def _build_program():
    import concourse.tile as tile
    import concourse.mybir as mybir
    from concourse import bacc

    f32 = mybir.dt.float32
    bf = mybir.dt.bfloat16
    fp8 = mybir.dt.float8e4
    AFT = mybir.ActivationFunctionType
    DR = mybir.MatmulPerfMode.DoubleRow

    edt = fp8 if ATTN_FP8 else bf

    nc = bacc.Bacc(
        "TRN2",
        target_bir_lowering=False,
        debug=False,
        enable_asserts=False,
        num_devices=NCORES,
    )
    xt_d = nc.dram_tensor("xt", [H, S], bf, kind="ExternalInput")
    wqk_d = nc.dram_tensor("wqk", [H, NCI * 128], bf, kind="ExternalInput")
    wv_d = nc.dram_tensor("wv", [H, HPC * 128], bf, kind="ExternalInput")
    wd_d = nc.dram_tensor("wd", [HPC * 128, H], bf, kind="ExternalInput")
    bqk_d = nc.dram_tensor("bqk", [128, NCI], f32, kind="ExternalInput")
    alibi_d = nc.dram_tensor("alibi", [128, HPC * N_KT], f32, kind="ExternalInput")
    tri_d = nc.dram_tensor("tri", [128, 128], edt, kind="ExternalInput")
    ztri_d = nc.dram_tensor("ztri", [128, 256], edt, kind="ExternalInput")
    out_d = nc.dram_tensor("out", [S, H], bf, kind="ExternalOutput")

    xt_r = xt_d.rearrange("(ho p) s -> p ho s", p=128)        # [128,16,2048]
    wqk_r = wqk_d.rearrange("(ho p) c -> p ho c", p=128)      # [128,16,1024]
    wv_r = wv_d.rearrange("(ho p) c -> p ho c", p=128)        # [128,16,512]
    wd_r = wd_d.rearrange("(co p) h -> p co h", p=128)        # [128,4,2048]
    out_r = out_d.rearrange("(so p) h -> p so h", p=128)      # [128,16,2048]

    with tile.TileContext(nc) as tc:
        with (
            tc.tile_pool(name="singles", bufs=1) as singles,
            tc.tile_pool(name="wstream", bufs=4) as wstream,
            tc.tile_pool(name="epool", bufs=8) as epool,
            tc.tile_pool(name="rpool", bufs=2) as rpool,
            tc.tile_pool(name="outstage", bufs=4) as outstage,
            tc.tile_pool(name="ps_qk", bufs=2, space="PSUM") as ps_qk,
            tc.tile_pool(name="ps_st", bufs=3, space="PSUM") as ps_st,
            tc.tile_pool(name="ps_ctx", bufs=2, space="PSUM") as ps_ctx,
            tc.tile_pool(name="ps_sums", bufs=1, space="PSUM") as ps_sums,
        ):
            # ---------------- DMA order tuned for startup ----------------
            # first QK unit depends only on w0 + xt slice 0; V on wv as well
            wtiles = {}
            wt = wstream.tile([128, N_HT, 128], bf, tag="wstream", name="wqk_0")
            nc.sync.dma_start(out=wt, in_=wqk_r[:, :, 0:128])
            wtiles[0] = wt
            xt_sb = singles.tile([128, N_HT, S], bf, tag="xt_sb", name="xt_sb")
            nc.sync.dma_start(out=xt_sb[:, :, 0:QSL], in_=xt_r[:, :, 0:QSL])
            bqk_sb = singles.tile([128, NCI], f32, tag="bqk_sb", name="bqk_sb")
            nc.sync.dma_start(out=bqk_sb, in_=bqk_d[:])
            wt = wstream.tile([128, N_HT, 128], bf, tag="wstream", name="wqk_1")
            nc.sync.dma_start(out=wt, in_=wqk_r[:, :, 128:256])
            wtiles[1] = wt
            wv_sb = singles.tile([128, N_HT, HPC * 128], bf, tag="wv_sb", name="wv_sb")
            nc.sync.dma_start(out=wv_sb, in_=wv_r)
            alibi_sb = singles.tile([128, HPC * N_KT], f32, tag="alibi_sb", name="alibi_sb")
            nc.sync.dma_start(out=alibi_sb, in_=alibi_d[:])
            tri_sb = singles.tile([128, 128], edt, tag="tri_sb", name="tri_sb")
            nc.sync.dma_start(out=tri_sb, in_=tri_d[:])
            ztri_sb = singles.tile([128, 256], edt, tag="ztri_sb", name="ztri_sb")
            nc.sync.dma_start(out=ztri_sb, in_=ztri_d[:])
            for ss in range(1, N_QS):
                nc.sync.dma_start(
                    out=xt_sb[:, :, ss * QSL:(ss + 1) * QSL],
                    in_=xt_r[:, :, ss * QSL:(ss + 1) * QSL],
                )
            if ATTN_FP8:
                ones8 = singles.tile([128, 2, 128], fp8, tag="ones8", name="ones8")
                nc.vector.memset(ones8, 1.0)
            else:
                ones_bf = singles.tile([128, 128], bf, tag="ones_bf", name="ones_bf")
                nc.vector.memset(ones_bf, 1.0)

            qkt_sb = singles.tile([128, NCI, S], bf, tag="qkt_sb", name="qkt_sb")
            v_sb = singles.tile([128, N_KT, HPC * 128], edt, tag="v_sb", name="v_sb")
            ctx_sb = singles.tile([128, HPC, S], bf, tag="ctx_sb", name="ctx_sb")
            wd_sb = singles.tile([128, HPC, H], bf, tag="wd_sb", name="wd_sb")

            def qk_unit(ci, ss):
                """project q or k column tile ci for s-slice ss."""
                wt = wtiles[ci]
                ps = ps_qk.tile([128, QSL], f32, tag="qk", name=f"qk_{ci}_{ss}")
                for ht in range(N_HT):
                    nc.tensor.matmul(
                        ps,
                        lhsT=wt[:, ht, :],
                        rhs=xt_sb[:, ht, ss * QSL:(ss + 1) * QSL],
                        start=(ht == 0),
                        stop=(ht == N_HT - 1),
                    )
                nc.scalar.activation(
                    out=qkt_sb[:, ci, ss * QSL:(ss + 1) * QSL],
                    in_=ps,
                    func=AFT.Identity,
                    bias=bqk_sb[:, ci:ci + 1],
                    scale=1.0,
                )

            # earliest compute: first two QK units need only w0/w1 + xt slice 0
            qk_unit(0, 0)
            qk_unit(1, 0)

            # V projection (natural [s, d] layout), fp8 output when ATTN_FP8
            for sti in range(N_KT):
                psv = ps_st.tile([128, QSL], f32, tag="st", name=f"v_{sti}")
                for ht in range(N_HT):
                    nc.tensor.matmul(
                        psv,
                        lhsT=xt_sb[:, ht, sti * 128:(sti + 1) * 128],
                        rhs=wv_sb[:, ht, :],
                        start=(ht == 0),
                        stop=(ht == N_HT - 1),
                    )
                nc.vector.tensor_copy(out=v_sb[:, sti, :], in_=psv)

            # rest of head-0 QK
            for ss in range(1, N_QS):
                qk_unit(0, ss)
                qk_unit(1, ss)

            # wd arrives during attention of head 0
            nc.sync.dma_start(out=wd_sb, in_=wd_r)

            def attention_qs_fp8(h, qs):
                q_ci, k_ci = 2 * h, 2 * h + 1
                ctx_ps = ps_ctx.tile([128, QSL], f32, tag="ctxps", name=f"ctx_{h}_{qs}")
                sums_ps = ps_sums.tile([128, QSL], f32, tag="sumsps", name=f"sums_{h}_{qs}")
                # (kt_even, col offset, width, masks=[(slot, tile, mw)])
                pairs = [(2 * t, 0, QSL, []) for t in range(2 * qs)]
                dm = [(0, tri_sb, 128), (1, ztri_sb, 256)]
                pairs += [(4 * qs, 0, QSL, dm), (4 * qs + 2, 256, 256, dm)]
                np_ = len(pairs)
                for i, (ka, c, w, masks) in enumerate(pairs):
                    e2 = epool.tile([128, 2, QSL], fp8, tag="etile", name=f"e_{h}_{qs}_{ka}")
                    for slot in range(2):
                        kt = ka + slot
                        st = ps_st.tile([128, QSL], f32, tag="st", name=f"st_{h}_{qs}_{kt}")
                        nc.tensor.matmul(
                            st[:, 0:w],
                            lhsT=qkt_sb[:, k_ci, kt * KTL:(kt + 1) * KTL],
                            rhs=qkt_sb[:, q_ci, qs * QSL + c:(qs + 1) * QSL],
                            start=True,
                            stop=True,
                        )
                        nc.scalar.activation(
                            out=e2[:, slot, 0:w],
                            in_=st[:, 0:w],
                            func=AFT.Exp,
                            bias=alibi_sb[:, h * N_KT + kt:h * N_KT + kt + 1],
                            scale=1.0,
                        )
                    for slot, mt, mw in masks:
                        nc.vector.tensor_mul(
                            out=e2[:, slot, 0:mw], in0=e2[:, slot, 0:mw], in1=mt[:, 0:mw]
                        )
                    first, last = (i == 0), (i == np_ - 1)
                    nc.tensor.matmul(
                        ctx_ps[:, c:QSL],
                        lhsT=v_sb[:, ka:ka + 2, h * 128:(h + 1) * 128],
                        rhs=e2[:, :, 0:w],
                        start=first,
                        stop=last,
                        perf_mode=DR,
                        skip_group_check=True,
                    )
                    nc.tensor.matmul(
                        sums_ps[:, c:QSL],
                        lhsT=ones8,
                        rhs=e2[:, :, 0:w],
                        start=first,
                        stop=last,
                        perf_mode=DR,
                        skip_group_check=True,
                    )
                recipb = rpool.tile([128, QSL], f32, tag="recipb", name=f"recipb_{h}_{qs}")
                nc.vector.reciprocal_approx_fast(out=recipb, in_=sums_ps)
                nc.vector.tensor_mul(
                    out=ctx_sb[:, h, qs * QSL:(qs + 1) * QSL],
                    in0=ctx_ps,
                    in1=recipb,
                )

            def attention_qs_bf16(h, qs):
                q_ci, k_ci = 2 * h, 2 * h + 1
                ctx_ps = ps_ctx.tile([128, QSL], f32, tag="ctxps", name=f"ctx_{h}_{qs}")
                sums_ps = ps_sums.tile([128, QSL], f32, tag="sumsps", name=f"sums_{h}_{qs}")
                blocks = [(kt, 0, QSL) for kt in range(4 * qs)]
                blocks += [(4 * qs + j, 128 * j, QSL - 128 * j) for j in range(4)]
                nb = len(blocks)
                for i, (kt, c, w) in enumerate(blocks):
                    diag = kt >= 4 * qs
                    st = ps_st.tile([128, QSL], f32, tag="st", name=f"st_{h}_{qs}_{kt}")
                    nc.tensor.matmul(
                        st[:, 0:w],
                        lhsT=qkt_sb[:, k_ci, kt * KTL:(kt + 1) * KTL],
                        rhs=qkt_sb[:, q_ci, qs * QSL + c:(qs + 1) * QSL],
                        start=True,
                        stop=True,
                    )
                    e_t = epool.tile([128, 2, QSL], bf, tag="etile", name=f"e_{h}_{qs}_{kt}")
                    nc.scalar.activation(
                        out=e_t[:, 0, 0:w],
                        in_=st[:, 0:w],
                        func=AFT.Exp,
                        bias=alibi_sb[:, h * N_KT + kt:h * N_KT + kt + 1],
                        scale=1.0,
                    )
                    if diag:
                        nc.vector.tensor_mul(
                            out=e_t[:, 0, 0:128], in0=e_t[:, 0, 0:128], in1=tri_sb
                        )
                    first, last = (i == 0), (i == nb - 1)
                    nc.tensor.matmul(
                        ctx_ps[:, c:QSL],
                        lhsT=v_sb[:, kt, h * 128:(h + 1) * 128],
                        rhs=e_t[:, 0, 0:w],
                        start=first,
                        stop=last,
                        skip_group_check=True,
                    )
                    nc.tensor.matmul(
                        sums_ps[:, c:QSL],
                        lhsT=ones_bf,
                        rhs=e_t[:, 0, 0:w],
                        start=first,
                        stop=last,
                        skip_group_check=True,
                    )
                recipb = rpool.tile([128, QSL], f32, tag="recipb", name=f"recipb_{h}_{qs}")
                nc.vector.reciprocal_approx_fast(out=recipb, in_=sums_ps)
                nc.vector.tensor_mul(
                    out=ctx_sb[:, h, qs * QSL:(qs + 1) * QSL],
                    in0=ctx_ps,
                    in1=recipb,
                )

            attention_qs = attention_qs_fp8 if ATTN_FP8 else attention_qs_bf16

            def dense_so(so):
                """dense partial for token tile so: [128 s, 2048 h']."""
                for hs in range(4):
                    psd = ps_qk.tile([128, QSL], f32, tag="qk", name=f"d_{so}_{hs}")
                    for ct in range(HPC):
                        nc.tensor.matmul(
                            psd,
                            lhsT=ctx_sb[:, ct, so * 128:(so + 1) * 128],
                            rhs=wd_sb[:, ct, hs * QSL:(hs + 1) * QSL],
                            start=(ct == 0),
                            stop=(ct == HPC - 1),
                        )
                    ot = outstage.tile([128, QSL], bf, tag="ostage", name=f"o_{so}_{hs}")
                    nc.vector.tensor_copy(out=ot, in_=psd)
                    nc.sync.dma_start(
                        out=out_r[:, so, hs * QSL:(hs + 1) * QSL], in_=ot
                    )

            # ------------- per-head pipeline -------------
            for h in range(HPC):
                if h < HPC - 1:
                    for ci in (2 * h + 2, 2 * h + 3):
                        wt = wstream.tile([128, N_HT, 128], bf, tag="wstream", name=f"wqk_{ci}")
                        nc.sync.dma_start(out=wt, in_=wqk_r[:, :, ci * 128:(ci + 1) * 128])
                        wtiles[ci] = wt
                for qs in range(N_QS):
                    attention_qs(h, qs)
                    if h < HPC - 1:
                        qk_unit(2 * h + 2, qs)
                        qk_unit(2 * h + 3, qs)
                    else:
                        for so in range(4 * qs, 4 * qs + 4):
                            dense_so(so)

    nc.compile()
    return nc


def _prepare_core_inputs(inputs):
    hs = np.asarray(inputs["hidden_states"], np.float32)
    alibi = np.asarray(inputs["alibi"], np.float32).reshape(B, NH, S)
    W_qkv = np.asarray(inputs["W_qkv"], np.float32).reshape(H, NH, 3, HD)
    b_qkv = np.asarray(inputs["b_qkv"], np.float32).reshape(NH, 3, HD)
    W_dense = np.asarray(inputs["W_dense"], np.float32)

    mask = np.asarray(inputs["attention_mask"], bool)
    causal = np.triu(np.ones((S, S), bool), k=1)
    assert all(np.array_equal(mask[b, 0], causal) for b in range(mask.shape[0])), \
        "kernel hardcodes the causal mask"

    mdt = f8 if ATTN_FP8 else bf16
    xt = [np.ascontiguousarray(hs[b].T).astype(bf16) for b in range(B)]
    tri = np.tril(np.ones((128, 128), np.float32)).T.astype(mdt)  # allow[k,q]=q>=k
    ztri = np.concatenate(
        [np.zeros((128, 128), np.float32), np.tril(np.ones((128, 128), np.float32)).T],
        axis=1,
    ).astype(mdt)

    in_maps = []
    for c in range(NCORES):
        b, g = divmod(c, TP)
        heads = range(HPC * g, HPC * g + HPC)
        wqk = np.empty((H, NCI * 128), np.float32)
        bqk = np.empty((NCI, 128), np.float32)
        wv = np.empty((H, HPC * 128), np.float32)
        wd = np.empty((HPC * 128, H), np.float32)
        al = np.empty((128, HPC * N_KT), np.float32)
        for i, hh in enumerate(heads):
            wqk[:, (2 * i) * 128:(2 * i + 1) * 128] = W_qkv[:, hh, 0, :] * INV_NORM
            wqk[:, (2 * i + 1) * 128:(2 * i + 2) * 128] = W_qkv[:, hh, 1, :]
            bqk[2 * i] = b_qkv[hh, 0, :] * INV_NORM
            bqk[2 * i + 1] = b_qkv[hh, 1, :]
            wv[:, i * 128:(i + 1) * 128] = W_qkv[:, hh, 2, :]
            wd[i * 128:(i + 1) * 128, :] = W_dense[hh * HD:(hh + 1) * HD, :]
            al[:, i * N_KT:(i + 1) * N_KT] = (
                alibi[b, hh].reshape(N_KT, 128).T - EXP_SHIFT
            )
        in_maps.append({
            "xt": xt[b],
            "wqk": wqk.astype(bf16),
            "wv": wv.astype(bf16),
            "wd": wd.astype(bf16),
            "bqk": np.ascontiguousarray(bqk.T),
            "alibi": al,
            "tri": tri,
            "ztri": ztri,
        })
    return in_maps


def _run(inputs, trace=False, trace_cores=None):
    from concourse.bass_utils import run_bass_kernel_spmd

    in_maps = _prepare_core_inputs(inputs)
    key = ("v31", ATTN_FP8)
    if key not in _program_cache:
        _program_cache[key] = _build_program()
    nc = _program_cache[key]
    res = run_bass_kernel_spmd(
        nc,
        in_maps,
        core_ids=list(range(NCORES)),
        trace=trace,
        trace_cores=trace_cores,
    )

    residual = np.asarray(inputs["residual"], np.float32)
    b_dense = np.asarray(inputs["b_dense"], np.float32)
    b_qkv = np.asarray(inputs["b_qkv"], np.float32).reshape(NH, 3, HD)
    W_dense = np.asarray(inputs["W_dense"], np.float32)
    bv_full = b_qkv[:, 2, :].reshape(H)
    b_eff = b_dense + bv_full @ W_dense  # folded V bias

    out = np.empty((B, S, H), np.float32)
    for b in range(B):
        acc = res.results[b * TP + 0]["out"].astype(np.float32)
        for g in range(1, TP):
            acc += res.results[b * TP + g]["out"].astype(np.float32)
        out[b] = acc + b_eff[None, :] + residual[b]
    return out, res


def kernel(**inputs) -> np.ndarray:
    out, _ = _run(inputs, trace=False)
    return out


# revision 9
# speedup vs baseline: 1.0256x; 1.0256x over previous
"""BLOOM attention layer on 8 Trainium2 NeuronCores.

Sharding: tensor-parallel over heads (4 heads/core) x data-parallel over batch
(B=2), mesh [DP=2, TP=4].  Core c handles batch b=c//4, heads 4*(c%4)..+3.

v3.1 design:
  - bf16 matmuls for QKV projection, scores and dense (fp8 fails the accuracy
    gate there); fp8e4 DoubleRow matmuls for the attention ctx and softmax-sum
    reductions: exp is written to fp8 E pairs [128, 2, 512] and each DR matmul
    contracts two 128-key tiles at one bf16-matmul's cost.  V is stored fp8.
    Scores are shifted by -2 (folded into alibi) so E stays inside e4m3 range;
    the shift cancels in the softmax ratio.
  - Per-head pipeline: QK projection of head h+1 (and the dense partials for
    the last head) are interleaved into attention of head h, which hides the
    scalar-engine exp stream and removes phase-boundary stalls.
  - Causal structure hardcoded: full key pairs plus diagonal pairs at widths
    (512,512) and (256,256) via nested partial-region PSUM accumulation; the
    only mask tiles are a shared 128x128 triangle and a 128x256 zero|triangle.
  - Softmax normalize: reciprocal_approx_fast + tensor_mul (the exact DVE
    reciprocal costs 3.4us/tile).  V bias is folded out on the host
    (softmax rows sum to 1 -> + bv passes through; bv @ W_dense joins
    b_dense).  Dense partials leave as bf16; host sums the 4 TP partials.
"""

import numpy as np
import ml_dtypes

bf16 = ml_dtypes.bfloat16
f8 = ml_dtypes.float8_e4m3fn

B, S, H, NH = 2, 2048, 2048, 16
HD = H // NH  # 128
INV_NORM = 1.0 / float(np.sqrt(HD))
NCORES = 8
TP = 4
HPC = NH // TP  # 4 heads per core
QSL = 512
KTL = 128
N_QS = S // QSL   # 4
N_KT = S // KTL   # 16
N_HT = H // 128   # 16 contraction tiles
NCI = 2 * HPC     # 8 q/k column tiles

ATTN_FP8 = True   # fp8e4 DoubleRow for ctx & softmax sums
EXP_SHIFT = 2.0   # scores shift folded into alibi (cancels in softmax)

_program_cache: dict = {}


def _build_program():
    import concourse.tile as tile
    import concourse.mybir as mybir
    from concourse import bacc

    f32 = mybir.dt.float32
    bf = mybir.dt.bfloat16
    fp8 = mybir.dt.float8e4
    AFT = mybir.ActivationFunctionType
    DR = mybir.MatmulPerfMode.DoubleRow

    edt = fp8 if ATTN_FP8 else bf

    nc = bacc.Bacc(
        "TRN2",
        target_bir_lowering=False,
        debug=False,
        enable_asserts=False,
        num_devices=NCORES,
    )
    xt_d = nc.dram_tensor("xt", [H, S], bf, kind="ExternalInput")
    wqk_d = nc.dram_tensor("wqk", [H, NCI * 128], bf, kind="ExternalInput")
    wv_d = nc.dram_tensor("wv", [H, HPC * 128], bf, kind="ExternalInput")
    wd_d = nc.dram_tensor("wd", [HPC * 128, H], bf, kind="ExternalInput")
    bqk_d = nc.dram_tensor("bqk", [128, NCI], f32, kind="ExternalInput")
    alibi_d = nc.dram_tensor("alibi", [128, HPC * N_KT], f32, kind="ExternalInput")
    tri_d = nc.dram_tensor("tri", [128, 128], edt, kind="ExternalInput")
    ztri_d = nc.dram_tensor("ztri", [128, 256], edt, kind="ExternalInput")
    out_d = nc.dram_tensor("out", [S, H], bf, kind="ExternalOutput")

    xt_r = xt_d.rearrange("(ho p) s -> p ho s", p=128)        # [128,16,2048]
    wqk_r = wqk_d.rearrange("(ho p) c -> p ho c", p=128)      # [128,16,1024]
    wv_r = wv_d.rearrange("(ho p) c -> p ho c", p=128)        # [128,16,512]
    wd_r = wd_d.rearrange("(co p) h -> p co h", p=128)        # [128,4,2048]
    out_r = out_d.rearrange("(so p) h -> p so h", p=128)      # [128,16,2048]

    with tile.TileContext(nc) as tc:
        with (
            tc.tile_pool(name="singles", bufs=1) as singles,
            tc.tile_pool(name="wstream", bufs=4) as wstream,
            tc.tile_pool(name="epool", bufs=8) as epool,
            tc.tile_pool(name="rpool", bufs=2) as rpool,
            tc.tile_pool(name="outstage", bufs=4) as outstage,
            tc.tile_pool(name="ps_qk", bufs=2, space="PSUM") as ps_qk,
            tc.tile_pool(name="ps_st", bufs=3, space="PSUM") as ps_st,
            tc.tile_pool(name="ps_ctx", bufs=2, space="PSUM") as ps_ctx,
            tc.tile_pool(name="ps_sums", bufs=1, space="PSUM") as ps_sums,
        ):
            # ---------------- DMA order tuned for startup ----------------
            # first two QK units depend on w0/w1 + xt slice 0 halves; V on wv
            wtiles = {}
            for ci in range(2):
                wt = wstream.tile([128, N_HT, 128], bf, tag="wstream", name=f"wqk_{ci}")
                nc.sync.dma_start(out=wt, in_=wqk_r[:, :, ci * 128:(ci + 1) * 128])
                wtiles[ci] = wt
            xt_sb = singles.tile([128, N_HT, S], bf, tag="xt_sb", name="xt_sb")
            for hc in range(2):
                nc.sync.dma_start(
                    out=xt_sb[:, hc * 8:(hc + 1) * 8, 0:QSL],
                    in_=xt_r[:, hc * 8:(hc + 1) * 8, 0:QSL],
                )
            bqk_sb = singles.tile([128, NCI], f32, tag="bqk_sb", name="bqk_sb")
            nc.sync.dma_start(out=bqk_sb, in_=bqk_d[:])
            alibi_sb = singles.tile([128, HPC * N_KT], f32, tag="alibi_sb", name="alibi_sb")
            nc.sync.dma_start(out=alibi_sb, in_=alibi_d[:])
            tri_sb = singles.tile([128, 128], edt, tag="tri_sb", name="tri_sb")
            nc.sync.dma_start(out=tri_sb, in_=tri_d[:])
            ztri_sb = singles.tile([128, 256], edt, tag="ztri_sb", name="ztri_sb")
            nc.sync.dma_start(out=ztri_sb, in_=ztri_d[:])
            wv_sb = singles.tile([128, N_HT, HPC * 128], bf, tag="wv_sb", name="wv_sb")
            for hc in range(2):
                nc.sync.dma_start(
                    out=wv_sb[:, hc * 8:(hc + 1) * 8, :],
                    in_=wv_r[:, hc * 8:(hc + 1) * 8, :],
                )
            for ss in range(1, N_QS):
                nc.sync.dma_start(
                    out=xt_sb[:, :, ss * QSL:(ss + 1) * QSL],
                    in_=xt_r[:, :, ss * QSL:(ss + 1) * QSL],
                )
            if ATTN_FP8:
                ones8 = singles.tile([128, 2, 128], fp8, tag="ones8", name="ones8")
                nc.vector.memset(ones8, 1.0)
            else:
                ones_bf = singles.tile([128, 128], bf, tag="ones_bf", name="ones_bf")
                nc.vector.memset(ones_bf, 1.0)

            qkt_sb = singles.tile([128, NCI, S], bf, tag="qkt_sb", name="qkt_sb")
            v_sb = singles.tile([128, N_KT, HPC * 128], edt, tag="v_sb", name="v_sb")
            ctx_sb = singles.tile([128, HPC, S], bf, tag="ctx_sb", name="ctx_sb")
            wd_sb = singles.tile([128, HPC, H], bf, tag="wd_sb", name="wd_sb")

            def qk_unit(ci, ss):
                """project q or k column tile ci for s-slice ss."""
                wt = wtiles[ci]
                ps = ps_qk.tile([128, QSL], f32, tag="qk", name=f"qk_{ci}_{ss}")
                for ht in range(N_HT):
                    nc.tensor.matmul(
                        ps,
                        lhsT=wt[:, ht, :],
                        rhs=xt_sb[:, ht, ss * QSL:(ss + 1) * QSL],
                        start=(ht == 0),
                        stop=(ht == N_HT - 1),
                    )
                nc.scalar.activation(
                    out=qkt_sb[:, ci, ss * QSL:(ss + 1) * QSL],
                    in_=ps,
                    func=AFT.Identity,
                    bias=bqk_sb[:, ci:ci + 1],
                    scale=1.0,
                )

            # earliest compute: first two QK units need only w0/w1 + xt slice 0
            qk_unit(0, 0)
            qk_unit(1, 0)

            # V projection (natural [s, d] layout), fp8 output when ATTN_FP8
            for sti in range(N_KT):
                psv = ps_st.tile([128, QSL], f32, tag="st", name=f"v_{sti}")
                for ht in range(N_HT):
                    nc.tensor.matmul(
                        psv,
                        lhsT=xt_sb[:, ht, sti * 128:(sti + 1) * 128],
                        rhs=wv_sb[:, ht, :],
                        start=(ht == 0),
                        stop=(ht == N_HT - 1),
                    )
                nc.vector.tensor_copy(out=v_sb[:, sti, :], in_=psv)

            # rest of head-0 QK
            for ss in range(1, N_QS):
                qk_unit(0, ss)
                qk_unit(1, ss)

            # wd arrives during attention of head 0
            nc.sync.dma_start(out=wd_sb, in_=wd_r)

            def attention_qs_fp8(h, qs):
                q_ci, k_ci = 2 * h, 2 * h + 1
                ctx_ps = ps_ctx.tile([128, QSL], f32, tag="ctxps", name=f"ctx_{h}_{qs}")
                sums_ps = ps_sums.tile([128, QSL], f32, tag="sumsps", name=f"sums_{h}_{qs}")
                # (kt_even, col offset, width, masks=[(slot, tile, mw)])
                pairs = [(2 * t, 0, QSL, []) for t in range(2 * qs)]
                dm = [(0, tri_sb, 128), (1, ztri_sb, 256)]
                pairs += [(4 * qs, 0, QSL, dm), (4 * qs + 2, 256, 256, dm)]
                np_ = len(pairs)
                for i, (ka, c, w, masks) in enumerate(pairs):
                    e2 = epool.tile([128, 2, QSL], fp8, tag="etile", name=f"e_{h}_{qs}_{ka}")
                    for slot in range(2):
                        kt = ka + slot
                        st = ps_st.tile([128, QSL], f32, tag="st", name=f"st_{h}_{qs}_{kt}")
                        nc.tensor.matmul(
                            st[:, 0:w],
                            lhsT=qkt_sb[:, k_ci, kt * KTL:(kt + 1) * KTL],
                            rhs=qkt_sb[:, q_ci, qs * QSL + c:(qs + 1) * QSL],
                            start=True,
                            stop=True,
                        )
                        nc.scalar.activation(
                            out=e2[:, slot, 0:w],
                            in_=st[:, 0:w],
                            func=AFT.Exp,
                            bias=alibi_sb[:, h * N_KT + kt:h * N_KT + kt + 1],
                            scale=1.0,
                        )
                    for slot, mt, mw in masks:
                        nc.vector.tensor_mul(
                            out=e2[:, slot, 0:mw], in0=e2[:, slot, 0:mw], in1=mt[:, 0:mw]
                        )
                    first, last = (i == 0), (i == np_ - 1)
                    nc.tensor.matmul(
                        ctx_ps[:, c:QSL],
                        lhsT=v_sb[:, ka:ka + 2, h * 128:(h + 1) * 128],
                        rhs=e2[:, :, 0:w],
                        start=first,
                        stop=last,
                        perf_mode=DR,
                        skip_group_check=True,
                    )
                    nc.tensor.matmul(
                        sums_ps[:, c:QSL],
                        lhsT=ones8,
                        rhs=e2[:, :, 0:w],
                        start=first,
                        stop=last,
                        perf_mode=DR,
                        skip_group_check=True,
                    )
                recipb = rpool.tile([128, QSL], f32, tag="recipb", name=f"recipb_{h}_{qs}")
                nc.vector.reciprocal_approx_fast(out=recipb, in_=sums_ps)
                nc.vector.tensor_mul(
                    out=ctx_sb[:, h, qs * QSL:(qs + 1) * QSL],
                    in0=ctx_ps,
                    in1=recipb,
                )

            def attention_qs_bf16(h, qs):
                q_ci, k_ci = 2 * h, 2 * h + 1
                ctx_ps = ps_ctx.tile([128, QSL], f32, tag="ctxps", name=f"ctx_{h}_{qs}")
                sums_ps = ps_sums.tile([128, QSL], f32, tag="sumsps", name=f"sums_{h}_{qs}")
                blocks = [(kt, 0, QSL) for kt in range(4 * qs)]
                blocks += [(4 * qs + j, 128 * j, QSL - 128 * j) for j in range(4)]
                nb = len(blocks)
                for i, (kt, c, w) in enumerate(blocks):
                    diag = kt >= 4 * qs
                    st = ps_st.tile([128, QSL], f32, tag="st", name=f"st_{h}_{qs}_{kt}")
                    nc.tensor.matmul(
                        st[:, 0:w],
                        lhsT=qkt_sb[:, k_ci, kt * KTL:(kt + 1) * KTL],
                        rhs=qkt_sb[:, q_ci, qs * QSL + c:(qs + 1) * QSL],
                        start=True,
                        stop=True,
                    )
                    e_t = epool.tile([128, 2, QSL], bf, tag="etile", name=f"e_{h}_{qs}_{kt}")
                    nc.scalar.activation(
                        out=e_t[:, 0, 0:w],
                        in_=st[:, 0:w],
                        func=AFT.Exp,
                        bias=alibi_sb[:, h * N_KT + kt:h * N_KT + kt + 1],
                        scale=1.0,
                    )
                    if diag:
                        nc.vector.tensor_mul(
                            out=e_t[:, 0, 0:128], in0=e_t[:, 0, 0:128], in1=tri_sb
                        )
                    first, last = (i == 0), (i == nb - 1)
                    nc.tensor.matmul(
                        ctx_ps[:, c:QSL],
                        lhsT=v_sb[:, kt, h * 128:(h + 1) * 128],
                        rhs=e_t[:, 0, 0:w],
                        start=first,
                        stop=last,
                        skip_group_check=True,
                    )
                    nc.tensor.matmul(
                        sums_ps[:, c:QSL],
                        lhsT=ones_bf,
                        rhs=e_t[:, 0, 0:w],
                        start=first,
                        stop=last,
                        skip_group_check=True,
                    )
                recipb = rpool.tile([128, QSL], f32, tag="recipb", name=f"recipb_{h}_{qs}")
                nc.vector.reciprocal_approx_fast(out=recipb, in_=sums_ps)
                nc.vector.tensor_mul(
                    out=ctx_sb[:, h, qs * QSL:(qs + 1) * QSL],
                    in0=ctx_ps,
                    in1=recipb,
                )

            attention_qs = attention_qs_fp8 if ATTN_FP8 else attention_qs_bf16

            def dense_so(so):
                """dense partial for token tile so: [128 s, 2048 h']."""
                for hs in range(4):
                    psd = ps_qk.tile([128, QSL], f32, tag="qk", name=f"d_{so}_{hs}")
                    for ct in range(HPC):
                        nc.tensor.matmul(
                            psd,
                            lhsT=ctx_sb[:, ct, so * 128:(so + 1) * 128],
                            rhs=wd_sb[:, ct, hs * QSL:(hs + 1) * QSL],
                            start=(ct == 0),
                            stop=(ct == HPC - 1),
                        )
                    ot = outstage.tile([128, QSL], bf, tag="ostage", name=f"o_{so}_{hs}")
                    nc.vector.tensor_copy(out=ot, in_=psd)
                    nc.sync.dma_start(
                        out=out_r[:, so, hs * QSL:(hs + 1) * QSL], in_=ot
                    )

            # ------------- per-head pipeline -------------
            for h in range(HPC):
                if h < HPC - 1:
                    for ci in (2 * h + 2, 2 * h + 3):
                        wt = wstream.tile([128, N_HT, 128], bf, tag="wstream", name=f"wqk_{ci}")
                        nc.sync.dma_start(out=wt, in_=wqk_r[:, :, ci * 128:(ci + 1) * 128])
                        wtiles[ci] = wt
                for qs in range(N_QS):
                    attention_qs(h, qs)
                    if h < HPC - 1:
                        qk_unit(2 * h + 2, qs)
                        qk_unit(2 * h + 3, qs)
                    else:
                        for so in range(4 * qs, 4 * qs + 4):
                            dense_so(so)

    nc.compile()
    return nc


def _prepare_core_inputs(inputs):
    hs = np.asarray(inputs["hidden_states"], np.float32)
    alibi = np.asarray(inputs["alibi"], np.float32).reshape(B, NH, S)
    W_qkv = np.asarray(inputs["W_qkv"], np.float32).reshape(H, NH, 3, HD)
    b_qkv = np.asarray(inputs["b_qkv"], np.float32).reshape(NH, 3, HD)
    W_dense = np.asarray(inputs["W_dense"], np.float32)

    mask = np.asarray(inputs["attention_mask"], bool)
    causal = np.triu(np.ones((S, S), bool), k=1)
    assert all(np.array_equal(mask[b, 0], causal) for b in range(mask.shape[0])), \
        "kernel hardcodes the causal mask"

    mdt = f8 if ATTN_FP8 else bf16
    xt = [np.ascontiguousarray(hs[b].T).astype(bf16) for b in range(B)]
    tri = np.tril(np.ones((128, 128), np.float32)).T.astype(mdt)  # allow[k,q]=q>=k
    ztri = np.concatenate(
        [np.zeros((128, 128), np.float32), np.tril(np.ones((128, 128), np.float32)).T],
        axis=1,
    ).astype(mdt)

    in_maps = []
    for c in range(NCORES):
        b, g = divmod(c, TP)
        heads = range(HPC * g, HPC * g + HPC)
        wqk = np.empty((H, NCI * 128), np.float32)
        bqk = np.empty((NCI, 128), np.float32)
        wv = np.empty((H, HPC * 128), np.float32)
        wd = np.empty((HPC * 128, H), np.float32)
        al = np.empty((128, HPC * N_KT), np.float32)
        for i, hh in enumerate(heads):
            wqk[:, (2 * i) * 128:(2 * i + 1) * 128] = W_qkv[:, hh, 0, :] * INV_NORM
            wqk[:, (2 * i + 1) * 128:(2 * i + 2) * 128] = W_qkv[:, hh, 1, :]
            bqk[2 * i] = b_qkv[hh, 0, :] * INV_NORM
            bqk[2 * i + 1] = b_qkv[hh, 1, :]
            wv[:, i * 128:(i + 1) * 128] = W_qkv[:, hh, 2, :]
            wd[i * 128:(i + 1) * 128, :] = W_dense[hh * HD:(hh + 1) * HD, :]
            al[:, i * N_KT:(i + 1) * N_KT] = (
                alibi[b, hh].reshape(N_KT, 128).T - EXP_SHIFT
            )
        in_maps.append({
            "xt": xt[b],
            "wqk": wqk.astype(bf16),
            "wv": wv.astype(bf16),
            "wd": wd.astype(bf16),
            "bqk": np.ascontiguousarray(bqk.T),
            "alibi": al,
            "tri": tri,
            "ztri": ztri,
        })
    return in_maps


def _run(inputs, trace=False, trace_cores=None):
    from concourse.bass_utils import run_bass_kernel_spmd

    in_maps = _prepare_core_inputs(inputs)
    key = ("v31", ATTN_FP8)
    if key not in _program_cache:
        _program_cache[key] = _build_program()
    nc = _program_cache[key]
    res = run_bass_kernel_spmd(
        nc,
        in_maps,
        core_ids=list(range(NCORES)),
        trace=trace,
        trace_cores=trace_cores,
    )

    residual = np.asarray(inputs["residual"], np.float32)
    b_dense = np.asarray(inputs["b_dense"], np.float32)
    b_qkv = np.asarray(inputs["b_qkv"], np.float32).reshape(NH, 3, HD)
    W_dense = np.asarray(inputs["W_dense"], np.float32)
    bv_full = b_qkv[:, 2, :].reshape(H)
    b_eff = b_dense + bv_full @ W_dense  # folded V bias

    out = np.empty((B, S, H), np.float32)
    for b in range(B):
        acc = res.results[b * TP + 0]["out"].astype(np.float32)
        for g in range(1, TP):
            acc += res.results[b * TP + g]["out"].astype(np.float32)
        out[b] = acc + b_eff[None, :] + residual[b]
    return out, res


def kernel(**inputs) -> np.ndarray:
    out, _ = _run(inputs, trace=False)
    return out


# revision 10
# speedup vs baseline: 1.0349x; 1.0091x over previous
"""BLOOM attention layer on 8 Trainium2 NeuronCores.

Sharding: tensor-parallel over heads (4 heads/core) x data-parallel over batch
(B=2), mesh [DP=2, TP=4].  Core c handles batch b=c//4, heads 4*(c%4)..+3.

v3.1 design:
  - bf16 matmuls for QKV projection, scores and dense (fp8 fails the accuracy
    gate there); fp8e4 DoubleRow matmuls for the attention ctx and softmax-sum
    reductions: exp is written to fp8 E pairs [128, 2, 512] and each DR matmul
    contracts two 128-key tiles at one bf16-matmul's cost.  V is stored fp8.
    Scores are shifted by -2 (folded into alibi) so E stays inside e4m3 range;
    the shift cancels in the softmax ratio.
  - Per-head pipeline: QK projection of head h+1 (and the dense partials for
    the last head) are interleaved into attention of head h, which hides the
    scalar-engine exp stream and removes phase-boundary stalls.
  - Causal structure hardcoded: full key pairs plus diagonal pairs at widths
    (512,512) and (256,256) via nested partial-region PSUM accumulation; the
    only mask tiles are a shared 128x128 triangle and a 128x256 zero|triangle.
  - Softmax normalize: reciprocal_approx_fast + tensor_mul (the exact DVE
    reciprocal costs 3.4us/tile).  V bias is folded out on the host
    (softmax rows sum to 1 -> + bv passes through; bv @ W_dense joins
    b_dense).  Dense partials leave as bf16; host sums the 4 TP partials.
"""

import numpy as np
import ml_dtypes

bf16 = ml_dtypes.bfloat16
f8 = ml_dtypes.float8_e4m3fn

B, S, H, NH = 2, 2048, 2048, 16
HD = H // NH  # 128
INV_NORM = 1.0 / float(np.sqrt(HD))
NCORES = 8
TP = 4
HPC = NH // TP  # 4 heads per core
QSL = 512
KTL = 128
N_QS = S // QSL   # 4
N_KT = S // KTL   # 16
N_HT = H // 128   # 16 contraction tiles
NCI = 2 * HPC     # 8 q/k column tiles

ATTN_FP8 = True   # fp8e4 DoubleRow for ctx & softmax sums
EXP_SHIFT = 2.0   # scores shift folded into alibi (cancels in softmax)

_program_cache: dict = {}


def _build_program():
    import concourse.tile as tile
    import concourse.mybir as mybir
    from concourse import bacc

    f32 = mybir.dt.float32
    bf = mybir.dt.bfloat16
    fp8 = mybir.dt.float8e4
    AFT = mybir.ActivationFunctionType
    DR = mybir.MatmulPerfMode.DoubleRow

    edt = fp8 if ATTN_FP8 else bf

    nc = bacc.Bacc(
        "TRN2",
        target_bir_lowering=False,
        debug=False,
        enable_asserts=False,
        num_devices=NCORES,
    )
    xt_d = nc.dram_tensor("xt", [H, S], bf, kind="ExternalInput")
    wqk_d = nc.dram_tensor("wqk", [H, NCI * 128], bf, kind="ExternalInput")
    wv_d = nc.dram_tensor("wv", [H, HPC * 128], bf, kind="ExternalInput")
    wd_d = nc.dram_tensor("wd", [HPC * 128, H], bf, kind="ExternalInput")
    bqk_d = nc.dram_tensor("bqk", [128, NCI], f32, kind="ExternalInput")
    alibi_d = nc.dram_tensor("alibi", [128, HPC * N_KT], f32, kind="ExternalInput")
    tri_d = nc.dram_tensor("tri", [128, 128], edt, kind="ExternalInput")
    ztri_d = nc.dram_tensor("ztri", [128, 256], edt, kind="ExternalInput")
    out_d = nc.dram_tensor("out", [S, H], bf, kind="ExternalOutput")

    xt_r = xt_d.rearrange("(ho p) s -> p ho s", p=128)        # [128,16,2048]
    wqk_r = wqk_d.rearrange("(ho p) c -> p ho c", p=128)      # [128,16,1024]
    wv_r = wv_d.rearrange("(ho p) c -> p ho c", p=128)        # [128,16,512]
    wd_r = wd_d.rearrange("(co p) h -> p co h", p=128)        # [128,4,2048]
    out_r = out_d.rearrange("(so p) h -> p so h", p=128)      # [128,16,2048]

    with tile.TileContext(nc) as tc:
        with (
            tc.tile_pool(name="singles", bufs=1) as singles,
            tc.tile_pool(name="wstream", bufs=4) as wstream,
            tc.tile_pool(name="epool", bufs=8) as epool,
            tc.tile_pool(name="rpool", bufs=2) as rpool,
            tc.tile_pool(name="outstage", bufs=4) as outstage,
            tc.tile_pool(name="ps_qk", bufs=2, space="PSUM") as ps_qk,
            tc.tile_pool(name="ps_st", bufs=3, space="PSUM") as ps_st,
            tc.tile_pool(name="ps_ctx", bufs=2, space="PSUM") as ps_ctx,
            tc.tile_pool(name="ps_sums", bufs=1, space="PSUM") as ps_sums,
        ):
            # ---------------- DMA order tuned for startup ----------------
            # first two QK units depend on w0/w1 + xt slice 0 halves; V on wv
            wtiles = {}
            for ci in range(2):
                wt = wstream.tile([128, N_HT, 128], bf, tag="wstream", name=f"wqk_{ci}")
                nc.sync.dma_start(out=wt, in_=wqk_r[:, :, ci * 128:(ci + 1) * 128])
                wtiles[ci] = wt
            xt_sb = singles.tile([128, N_HT, S], bf, tag="xt_sb", name="xt_sb")
            for hc in range(2):
                nc.sync.dma_start(
                    out=xt_sb[:, hc * 8:(hc + 1) * 8, 0:QSL],
                    in_=xt_r[:, hc * 8:(hc + 1) * 8, 0:QSL],
                )
            bqk_sb = singles.tile([128, NCI], f32, tag="bqk_sb", name="bqk_sb")
            nc.sync.dma_start(out=bqk_sb, in_=bqk_d[:])
            alibi_sb = singles.tile([128, HPC * N_KT], f32, tag="alibi_sb", name="alibi_sb")
            nc.sync.dma_start(out=alibi_sb, in_=alibi_d[:])
            tri_sb = singles.tile([128, 128], edt, tag="tri_sb", name="tri_sb")
            nc.sync.dma_start(out=tri_sb, in_=tri_d[:])
            ztri_sb = singles.tile([128, 256], edt, tag="ztri_sb", name="ztri_sb")
            nc.sync.dma_start(out=ztri_sb, in_=ztri_d[:])
            wv_sb = singles.tile([128, N_HT, HPC * 128], bf, tag="wv_sb", name="wv_sb")
            for hc in range(2):
                nc.sync.dma_start(
                    out=wv_sb[:, hc * 8:(hc + 1) * 8, :],
                    in_=wv_r[:, hc * 8:(hc + 1) * 8, :],
                )
            for ss in range(1, N_QS):
                nc.sync.dma_start(
                    out=xt_sb[:, :, ss * QSL:(ss + 1) * QSL],
                    in_=xt_r[:, :, ss * QSL:(ss + 1) * QSL],
                )
            if ATTN_FP8:
                ones8 = singles.tile([128, 2, 128], fp8, tag="ones8", name="ones8")
                nc.vector.memset(ones8, 1.0)
            else:
                ones_bf = singles.tile([128, 128], bf, tag="ones_bf", name="ones_bf")
                nc.vector.memset(ones_bf, 1.0)

            qkt_sb = singles.tile([128, NCI, S], bf, tag="qkt_sb", name="qkt_sb")
            v_sb = singles.tile([128, N_KT, HPC * 128], edt, tag="v_sb", name="v_sb")
            ctx_sb = singles.tile([128, HPC, S], bf, tag="ctx_sb", name="ctx_sb")
            wd_sb = singles.tile([128, HPC, H], bf, tag="wd_sb", name="wd_sb")

            def qk_unit(ci, ss):
                """project q or k column tile ci for s-slice ss."""
                wt = wtiles[ci]
                ps = ps_qk.tile([128, QSL], f32, tag="qk", name=f"qk_{ci}_{ss}")
                for ht in range(N_HT):
                    nc.tensor.matmul(
                        ps,
                        lhsT=wt[:, ht, :],
                        rhs=xt_sb[:, ht, ss * QSL:(ss + 1) * QSL],
                        start=(ht == 0),
                        stop=(ht == N_HT - 1),
                    )
                nc.scalar.activation(
                    out=qkt_sb[:, ci, ss * QSL:(ss + 1) * QSL],
                    in_=ps,
                    func=AFT.Identity,
                    bias=bqk_sb[:, ci:ci + 1],
                    scale=1.0,
                )

            # earliest compute: first two QK units need only w0/w1 + xt slice 0
            qk_unit(0, 0)
            qk_unit(1, 0)

            # V projection (natural [s, d] layout), fp8 output when ATTN_FP8
            for sti in range(N_KT):
                psv = ps_st.tile([128, QSL], f32, tag="st", name=f"v_{sti}")
                for ht in range(N_HT):
                    nc.tensor.matmul(
                        psv,
                        lhsT=xt_sb[:, ht, sti * 128:(sti + 1) * 128],
                        rhs=wv_sb[:, ht, :],
                        start=(ht == 0),
                        stop=(ht == N_HT - 1),
                    )
                nc.vector.tensor_copy(out=v_sb[:, sti, :], in_=psv)

            # rest of head-0 QK
            for ss in range(1, N_QS):
                qk_unit(0, ss)
                qk_unit(1, ss)

            # wd arrives during attention of head 0
            nc.sync.dma_start(out=wd_sb, in_=wd_r)

            def attention_qs_fp8(h, qs):
                q_ci, k_ci = 2 * h, 2 * h + 1
                ctx_ps = ps_ctx.tile([128, QSL], f32, tag="ctxps", name=f"ctx_{h}_{qs}")
                sums_ps = ps_sums.tile([128, QSL], f32, tag="sumsps", name=f"sums_{h}_{qs}")
                # (kt_even, col offset, width, masks=[(slot, tile, mw)])
                pairs = [(2 * t, 0, QSL, []) for t in range(2 * qs)]
                dm = [(0, tri_sb, 128), (1, ztri_sb, 256)]
                pairs += [(4 * qs, 0, QSL, dm), (4 * qs + 2, 256, 256, dm)]
                np_ = len(pairs)
                for i, (ka, c, w, masks) in enumerate(pairs):
                    e2 = epool.tile([128, 2, QSL], fp8, tag="etile", name=f"e_{h}_{qs}_{ka}")
                    for slot in range(2):
                        kt = ka + slot
                        st = ps_st.tile([128, QSL], f32, tag="st", name=f"st_{h}_{qs}_{kt}")
                        nc.tensor.matmul(
                            st[:, 0:w],
                            lhsT=qkt_sb[:, k_ci, kt * KTL:(kt + 1) * KTL],
                            rhs=qkt_sb[:, q_ci, qs * QSL + c:(qs + 1) * QSL],
                            start=True,
                            stop=True,
                        )
                        nc.scalar.activation(
                            out=e2[:, slot, 0:w],
                            in_=st[:, 0:w],
                            func=AFT.Exp,
                            bias=alibi_sb[:, h * N_KT + kt:h * N_KT + kt + 1],
                            scale=1.0,
                        )
                    for slot, mt, mw in masks:
                        nc.vector.tensor_mul(
                            out=e2[:, slot, 0:mw], in0=e2[:, slot, 0:mw], in1=mt[:, 0:mw]
                        )
                    first, last = (i == 0), (i == np_ - 1)
                    nc.tensor.matmul(
                        ctx_ps[:, c:QSL],
                        lhsT=v_sb[:, ka:ka + 2, h * 128:(h + 1) * 128],
                        rhs=e2[:, :, 0:w],
                        start=first,
                        stop=last,
                        perf_mode=DR,
                        skip_group_check=True,
                    )
                    nc.tensor.matmul(
                        sums_ps[:, c:QSL],
                        lhsT=ones8,
                        rhs=e2[:, :, 0:w],
                        start=first,
                        stop=last,
                        perf_mode=DR,
                        skip_group_check=True,
                    )
                recipb = rpool.tile([128, QSL], f32, tag="recipb", name=f"recipb_{h}_{qs}")
                nc.vector.reciprocal_approx_fast(out=recipb, in_=sums_ps)
                nc.vector.tensor_mul(
                    out=ctx_sb[:, h, qs * QSL:(qs + 1) * QSL],
                    in0=ctx_ps,
                    in1=recipb,
                )

            def attention_qs_bf16(h, qs):
                q_ci, k_ci = 2 * h, 2 * h + 1
                ctx_ps = ps_ctx.tile([128, QSL], f32, tag="ctxps", name=f"ctx_{h}_{qs}")
                sums_ps = ps_sums.tile([128, QSL], f32, tag="sumsps", name=f"sums_{h}_{qs}")
                blocks = [(kt, 0, QSL) for kt in range(4 * qs)]
                blocks += [(4 * qs + j, 128 * j, QSL - 128 * j) for j in range(4)]
                nb = len(blocks)
                for i, (kt, c, w) in enumerate(blocks):
                    diag = kt >= 4 * qs
                    st = ps_st.tile([128, QSL], f32, tag="st", name=f"st_{h}_{qs}_{kt}")
                    nc.tensor.matmul(
                        st[:, 0:w],
                        lhsT=qkt_sb[:, k_ci, kt * KTL:(kt + 1) * KTL],
                        rhs=qkt_sb[:, q_ci, qs * QSL + c:(qs + 1) * QSL],
                        start=True,
                        stop=True,
                    )
                    e_t = epool.tile([128, 2, QSL], bf, tag="etile", name=f"e_{h}_{qs}_{kt}")
                    nc.scalar.activation(
                        out=e_t[:, 0, 0:w],
                        in_=st[:, 0:w],
                        func=AFT.Exp,
                        bias=alibi_sb[:, h * N_KT + kt:h * N_KT + kt + 1],
                        scale=1.0,
                    )
                    if diag:
                        nc.vector.tensor_mul(
                            out=e_t[:, 0, 0:128], in0=e_t[:, 0, 0:128], in1=tri_sb
                        )
                    first, last = (i == 0), (i == nb - 1)
                    nc.tensor.matmul(
                        ctx_ps[:, c:QSL],
                        lhsT=v_sb[:, kt, h * 128:(h + 1) * 128],
                        rhs=e_t[:, 0, 0:w],
                        start=first,
                        stop=last,
                        skip_group_check=True,
                    )
                    nc.tensor.matmul(
                        sums_ps[:, c:QSL],
                        lhsT=ones_bf,
                        rhs=e_t[:, 0, 0:w],
                        start=first,
                        stop=last,
                        skip_group_check=True,
                    )
                recipb = rpool.tile([128, QSL], f32, tag="recipb", name=f"recipb_{h}_{qs}")
                nc.vector.reciprocal_approx_fast(out=recipb, in_=sums_ps)
                nc.vector.tensor_mul(
                    out=ctx_sb[:, h, qs * QSL:(qs + 1) * QSL],
                    in0=ctx_ps,
                    in1=recipb,
                )

            attention_qs = attention_qs_fp8 if ATTN_FP8 else attention_qs_bf16

            def dense_so(so):
                """dense partial for token tile so: [128 s, 2048 h']."""
                for hs in range(4):
                    psd = ps_qk.tile([128, QSL], f32, tag="qk", name=f"d_{so}_{hs}")
                    for ct in range(HPC):
                        nc.tensor.matmul(
                            psd,
                            lhsT=ctx_sb[:, ct, so * 128:(so + 1) * 128],
                            rhs=wd_sb[:, ct, hs * QSL:(hs + 1) * QSL],
                            start=(ct == 0),
                            stop=(ct == HPC - 1),
                        )
                    ot = outstage.tile([128, QSL], bf, tag="ostage", name=f"o_{so}_{hs}")
                    if hs % 2 == 0:
                        nc.vector.tensor_copy(out=ot, in_=psd)
                    else:
                        nc.scalar.copy(out=ot, in_=psd)
                    nc.sync.dma_start(
                        out=out_r[:, so, hs * QSL:(hs + 1) * QSL], in_=ot
                    )

            # ------------- per-head pipeline -------------
            for h in range(HPC):
                if h < HPC - 1:
                    for ci in (2 * h + 2, 2 * h + 3):
                        wt = wstream.tile([128, N_HT, 128], bf, tag="wstream", name=f"wqk_{ci}")
                        nc.sync.dma_start(out=wt, in_=wqk_r[:, :, ci * 128:(ci + 1) * 128])
                        wtiles[ci] = wt
                for qs in range(N_QS):
                    attention_qs(h, qs)
                    if h < HPC - 1:
                        qk_unit(2 * h + 2, qs)
                        qk_unit(2 * h + 3, qs)
                    else:
                        for so in range(4 * qs, 4 * qs + 4):
                            dense_so(so)

    nc.compile()
    return nc


def _prepare_core_inputs(inputs):
    hs = np.asarray(inputs["hidden_states"], np.float32)
    alibi = np.asarray(inputs["alibi"], np.float32).reshape(B, NH, S)
    W_qkv = np.asarray(inputs["W_qkv"], np.float32).reshape(H, NH, 3, HD)
    b_qkv = np.asarray(inputs["b_qkv"], np.float32).reshape(NH, 3, HD)
    W_dense = np.asarray(inputs["W_dense"], np.float32)

    mask = np.asarray(inputs["attention_mask"], bool)
    causal = np.triu(np.ones((S, S), bool), k=1)
    assert all(np.array_equal(mask[b, 0], causal) for b in range(mask.shape[0])), \
        "kernel hardcodes the causal mask"

    mdt = f8 if ATTN_FP8 else bf16
    xt = [np.ascontiguousarray(hs[b].T).astype(bf16) for b in range(B)]
    tri = np.tril(np.ones((128, 128), np.float32)).T.astype(mdt)  # allow[k,q]=q>=k
    ztri = np.concatenate(
        [np.zeros((128, 128), np.float32), np.tril(np.ones((128, 128), np.float32)).T],
        axis=1,
    ).astype(mdt)

    in_maps = []
    for c in range(NCORES):
        b, g = divmod(c, TP)
        heads = range(HPC * g, HPC * g + HPC)
        wqk = np.empty((H, NCI * 128), np.float32)
        bqk = np.empty((NCI, 128), np.float32)
        wv = np.empty((H, HPC * 128), np.float32)
        wd = np.empty((HPC * 128, H), np.float32)
        al = np.empty((128, HPC * N_KT), np.float32)
        for i, hh in enumerate(heads):
            wqk[:, (2 * i) * 128:(2 * i + 1) * 128] = W_qkv[:, hh, 0, :] * INV_NORM
            wqk[:, (2 * i + 1) * 128:(2 * i + 2) * 128] = W_qkv[:, hh, 1, :]
            bqk[2 * i] = b_qkv[hh, 0, :] * INV_NORM
            bqk[2 * i + 1] = b_qkv[hh, 1, :]
            wv[:, i * 128:(i + 1) * 128] = W_qkv[:, hh, 2, :]
            wd[i * 128:(i + 1) * 128, :] = W_dense[hh * HD:(hh + 1) * HD, :]
            al[:, i * N_KT:(i + 1) * N_KT] = (
                alibi[b, hh].reshape(N_KT, 128).T - EXP_SHIFT
            )
        in_maps.append({
            "xt": xt[b],
            "wqk": wqk.astype(bf16),
            "wv": wv.astype(bf16),
            "wd": wd.astype(bf16),
            "bqk": np.ascontiguousarray(bqk.T),
            "alibi": al,
            "tri": tri,
            "ztri": ztri,
        })
    return in_maps


def _run(inputs, trace=False, trace_cores=None):
    from concourse.bass_utils import run_bass_kernel_spmd

    in_maps = _prepare_core_inputs(inputs)
    key = ("v31", ATTN_FP8)
    if key not in _program_cache:
        _program_cache[key] = _build_program()
    nc = _program_cache[key]
    res = run_bass_kernel_spmd(
        nc,
        in_maps,
        core_ids=list(range(NCORES)),
        trace=trace,
        trace_cores=trace_cores,
    )

    residual = np.asarray(inputs["residual"], np.float32)
    b_dense = np.asarray(inputs["b_dense"], np.float32)
    b_qkv = np.asarray(inputs["b_qkv"], np.float32).reshape(NH, 3, HD)
    W_dense = np.asarray(inputs["W_dense"], np.float32)
    bv_full = b_qkv[:, 2, :].reshape(H)
    b_eff = b_dense + bv_full @ W_dense  # folded V bias

    out = np.empty((B, S, H), np.float32)
    for b in range(B):
        acc = res.results[b * TP + 0]["out"].astype(np.float32)
        for g in range(1, TP):
            acc += res.results[b * TP + g]["out"].astype(np.float32)
        out[b] = acc + b_eff[None, :] + residual[b]
    return out, res


def kernel(**inputs) -> np.ndarray:
    out, _ = _run(inputs, trace=False)
    return out


# revision 15
# speedup vs baseline: 1.0379x; 1.0029x over previous
"""BLOOM attention layer on 8 Trainium2 NeuronCores.

Sharding: tensor-parallel over heads (4 heads/core) x data-parallel over batch
(B=2), mesh [DP=2, TP=4].  Core c handles batch b=c//4, heads 4*(c%4)..+3.

v3.1 design:
  - bf16 matmuls for QKV projection, scores and dense (fp8 fails the accuracy
    gate there); fp8e4 DoubleRow matmuls for the attention ctx and softmax-sum
    reductions: exp is written to fp8 E pairs [128, 2, 512] and each DR matmul
    contracts two 128-key tiles at one bf16-matmul's cost.  V is stored fp8.
    Scores are shifted by -2 (folded into alibi) so E stays inside e4m3 range;
    the shift cancels in the softmax ratio.
  - Per-head pipeline: QK projection of head h+1 (and the dense partials for
    the last head) are interleaved into attention of head h, which hides the
    scalar-engine exp stream and removes phase-boundary stalls.
  - Causal structure hardcoded: full key pairs plus diagonal pairs at widths
    (512,512) and (256,256) via nested partial-region PSUM accumulation; the
    only mask tiles are a shared 128x128 triangle and a 128x256 zero|triangle.
  - Softmax normalize: reciprocal_approx_fast + tensor_mul (the exact DVE
    reciprocal costs 3.4us/tile).  V bias is folded out on the host
    (softmax rows sum to 1 -> + bv passes through; bv @ W_dense joins
    b_dense).  Dense partials leave as bf16; host sums the 4 TP partials.
"""

import numpy as np
import ml_dtypes

bf16 = ml_dtypes.bfloat16
f8 = ml_dtypes.float8_e4m3fn

B, S, H, NH = 2, 2048, 2048, 16
HD = H // NH  # 128
INV_NORM = 1.0 / float(np.sqrt(HD))
NCORES = 8
TP = 4
HPC = NH // TP  # 4 heads per core
QSL = 512
KTL = 128
N_QS = S // QSL   # 4
N_KT = S // KTL   # 16
N_HT = H // 128   # 16 contraction tiles
NCI = 2 * HPC     # 8 q/k column tiles

ATTN_FP8 = True   # fp8e4 DoubleRow for ctx & softmax sums
EXP_SHIFT = 2.0   # scores shift folded into alibi (cancels in softmax)

_program_cache: dict = {}


def _build_program():
    import concourse.tile as tile
    import concourse.mybir as mybir
    from concourse import bacc

    f32 = mybir.dt.float32
    bf = mybir.dt.bfloat16
    fp8 = mybir.dt.float8e4
    AFT = mybir.ActivationFunctionType
    DR = mybir.MatmulPerfMode.DoubleRow

    edt = fp8 if ATTN_FP8 else bf

    nc = bacc.Bacc(
        "TRN2",
        target_bir_lowering=False,
        debug=False,
        enable_asserts=False,
        num_devices=NCORES,
    )
    # startup-critical tensors are packed partition-major on the host so each
    # partition's DMA reads 4KB-contiguous segments instead of 256B strides
    xt_d = nc.dram_tensor("xt", [128, N_HT, S], bf, kind="ExternalInput")
    wqk_d = nc.dram_tensor("wqk", [128, NCI, N_HT, 128], bf, kind="ExternalInput")
    wv_d = nc.dram_tensor("wv", [128, N_HT, HPC * 128], bf, kind="ExternalInput")
    wd_d = nc.dram_tensor("wd", [HPC * 128, H], bf, kind="ExternalInput")
    bqk_d = nc.dram_tensor("bqk", [128, NCI], f32, kind="ExternalInput")
    alibi_d = nc.dram_tensor("alibi", [128, HPC * N_KT], f32, kind="ExternalInput")
    tri_d = nc.dram_tensor("tri", [128, 128], edt, kind="ExternalInput")
    ztri_d = nc.dram_tensor("ztri", [128, 256], edt, kind="ExternalInput")
    out_d = nc.dram_tensor("out", [S, H], bf, kind="ExternalOutput")

    xt_r = xt_d        # [128,16,2048]
    wv_r = wv_d        # [128,16,512]
    wd_r = wd_d.rearrange("(co p) h -> p co h", p=128)        # [128,4,2048]
    out_r = out_d.rearrange("(so p) h -> p so h", p=128)      # [128,16,2048]

    with tile.TileContext(nc) as tc:
        with (
            tc.tile_pool(name="singles", bufs=1) as singles,
            tc.tile_pool(name="wstream", bufs=4) as wstream,
            tc.tile_pool(name="epool", bufs=8) as epool,
            tc.tile_pool(name="rpool", bufs=2) as rpool,
            tc.tile_pool(name="outstage", bufs=4) as outstage,
            tc.tile_pool(name="ps_qk", bufs=2, space="PSUM") as ps_qk,
            tc.tile_pool(name="ps_st", bufs=3, space="PSUM") as ps_st,
            tc.tile_pool(name="ps_ctx", bufs=2, space="PSUM") as ps_ctx,
            tc.tile_pool(name="ps_sums", bufs=1, space="PSUM") as ps_sums,
        ):
            # ---------------- DMA order tuned for startup ----------------
            # first two QK units depend on w0/w1 + xt slice 0 halves; V on wv
            wtiles = {}
            for ci in range(2):
                wt = wstream.tile([128, N_HT, 128], bf, tag="wstream", name=f"wqk_{ci}")
                nc.sync.dma_start(out=wt, in_=wqk_d[:, ci])
                wtiles[ci] = wt
            xt_sb = singles.tile([128, N_HT, S], bf, tag="xt_sb", name="xt_sb")
            for hc in range(2):
                nc.sync.dma_start(
                    out=xt_sb[:, hc * 8:(hc + 1) * 8, 0:QSL],
                    in_=xt_r[:, hc * 8:(hc + 1) * 8, 0:QSL],
                )
            bqk_sb = singles.tile([128, NCI], f32, tag="bqk_sb", name="bqk_sb")
            nc.sync.dma_start(out=bqk_sb, in_=bqk_d[:])
            alibi_sb = singles.tile([128, HPC * N_KT], f32, tag="alibi_sb", name="alibi_sb")
            nc.sync.dma_start(out=alibi_sb, in_=alibi_d[:])
            tri_sb = singles.tile([128, 128], edt, tag="tri_sb", name="tri_sb")
            nc.sync.dma_start(out=tri_sb, in_=tri_d[:])
            ztri_sb = singles.tile([128, 256], edt, tag="ztri_sb", name="ztri_sb")
            nc.sync.dma_start(out=ztri_sb, in_=ztri_d[:])
            wv_sb = singles.tile([128, N_HT, HPC * 128], bf, tag="wv_sb", name="wv_sb")
            for hc in range(2):
                nc.sync.dma_start(
                    out=wv_sb[:, hc * 8:(hc + 1) * 8, :],
                    in_=wv_r[:, hc * 8:(hc + 1) * 8, :],
                )
            for ss in range(1, N_QS):
                nc.sync.dma_start(
                    out=xt_sb[:, :, ss * QSL:(ss + 1) * QSL],
                    in_=xt_r[:, :, ss * QSL:(ss + 1) * QSL],
                )
            if ATTN_FP8:
                ones8 = singles.tile([128, 2, 128], fp8, tag="ones8", name="ones8")
                nc.vector.memset(ones8, 1.0)
            else:
                ones_bf = singles.tile([128, 128], bf, tag="ones_bf", name="ones_bf")
                nc.vector.memset(ones_bf, 1.0)

            qkt_sb = singles.tile([128, NCI, S], bf, tag="qkt_sb", name="qkt_sb")
            v_sb = singles.tile([128, N_KT, HPC * 128], edt, tag="v_sb", name="v_sb")
            ctx_sb = singles.tile([128, HPC, S], bf, tag="ctx_sb", name="ctx_sb")
            wd_sb = singles.tile([128, HPC, H], bf, tag="wd_sb", name="wd_sb")

            def qk_unit(ci, ss):
                """project q or k column tile ci for s-slice ss."""
                wt = wtiles[ci]
                ps = ps_qk.tile([128, QSL], f32, tag="qk", name=f"qk_{ci}_{ss}")
                for ht in range(N_HT):
                    nc.tensor.matmul(
                        ps,
                        lhsT=wt[:, ht, :],
                        rhs=xt_sb[:, ht, ss * QSL:(ss + 1) * QSL],
                        start=(ht == 0),
                        stop=(ht == N_HT - 1),
                    )
                nc.scalar.activation(
                    out=qkt_sb[:, ci, ss * QSL:(ss + 1) * QSL],
                    in_=ps,
                    func=AFT.Identity,
                    bias=bqk_sb[:, ci:ci + 1],
                    scale=1.0,
                )

            # earliest compute: first two QK units need only w0/w1 + xt slice 0
            qk_unit(0, 0)
            qk_unit(1, 0)

            # V projection (natural [s, d] layout), fp8 output when ATTN_FP8
            for sti in range(N_KT):
                psv = ps_st.tile([128, QSL], f32, tag="st", name=f"v_{sti}")
                for ht in range(N_HT):
                    nc.tensor.matmul(
                        psv,
                        lhsT=xt_sb[:, ht, sti * 128:(sti + 1) * 128],
                        rhs=wv_sb[:, ht, :],
                        start=(ht == 0),
                        stop=(ht == N_HT - 1),
                    )
                nc.vector.tensor_copy(out=v_sb[:, sti, :], in_=psv)

            # rest of head-0 QK
            for ss in range(1, N_QS):
                qk_unit(0, ss)
                qk_unit(1, ss)

            # wd arrives during attention of head 0
            nc.sync.dma_start(out=wd_sb, in_=wd_r)

            def attention_qs_fp8(h, qs):
                q_ci, k_ci = 2 * h, 2 * h + 1
                ctx_ps = ps_ctx.tile([128, QSL], f32, tag="ctxps", name=f"ctx_{h}_{qs}")
                sums_ps = ps_sums.tile([128, QSL], f32, tag="sumsps", name=f"sums_{h}_{qs}")
                # (kt_even, col offset, width, masks=[(slot, tile, mw)])
                pairs = [(2 * t, 0, QSL, []) for t in range(2 * qs)]
                dm = [(0, tri_sb, 128), (1, ztri_sb, 256)]
                pairs += [(4 * qs, 0, QSL, dm), (4 * qs + 2, 256, 256, dm)]
                np_ = len(pairs)
                for i, (ka, c, w, masks) in enumerate(pairs):
                    e2 = epool.tile([128, 2, QSL], fp8, tag="etile", name=f"e_{h}_{qs}_{ka}")
                    for slot in range(2):
                        kt = ka + slot
                        st = ps_st.tile([128, QSL], f32, tag="st", name=f"st_{h}_{qs}_{kt}")
                        nc.tensor.matmul(
                            st[:, 0:w],
                            lhsT=qkt_sb[:, k_ci, kt * KTL:(kt + 1) * KTL],
                            rhs=qkt_sb[:, q_ci, qs * QSL + c:(qs + 1) * QSL],
                            start=True,
                            stop=True,
                        )
                        nc.scalar.activation(
                            out=e2[:, slot, 0:w],
                            in_=st[:, 0:w],
                            func=AFT.Exp,
                            bias=alibi_sb[:, h * N_KT + kt:h * N_KT + kt + 1],
                            scale=1.0,
                        )
                    for slot, mt, mw in masks:
                        nc.vector.tensor_mul(
                            out=e2[:, slot, 0:mw], in0=e2[:, slot, 0:mw], in1=mt[:, 0:mw]
                        )
                    first, last = (i == 0), (i == np_ - 1)
                    nc.tensor.matmul(
                        ctx_ps[:, c:QSL],
                        lhsT=v_sb[:, ka:ka + 2, h * 128:(h + 1) * 128],
                        rhs=e2[:, :, 0:w],
                        start=first,
                        stop=last,
                        perf_mode=DR,
                        skip_group_check=True,
                    )
                    nc.tensor.matmul(
                        sums_ps[:, c:QSL],
                        lhsT=ones8,
                        rhs=e2[:, :, 0:w],
                        start=first,
                        stop=last,
                        perf_mode=DR,
                        skip_group_check=True,
                    )
                recipb = rpool.tile([128, QSL], f32, tag="recipb", name=f"recipb_{h}_{qs}")
                nc.vector.reciprocal_approx_fast(out=recipb, in_=sums_ps)
                nc.vector.tensor_mul(
                    out=ctx_sb[:, h, qs * QSL:(qs + 1) * QSL],
                    in0=ctx_ps,
                    in1=recipb,
                )

            def attention_qs_bf16(h, qs):
                q_ci, k_ci = 2 * h, 2 * h + 1
                ctx_ps = ps_ctx.tile([128, QSL], f32, tag="ctxps", name=f"ctx_{h}_{qs}")
                sums_ps = ps_sums.tile([128, QSL], f32, tag="sumsps", name=f"sums_{h}_{qs}")
                blocks = [(kt, 0, QSL) for kt in range(4 * qs)]
                blocks += [(4 * qs + j, 128 * j, QSL - 128 * j) for j in range(4)]
                nb = len(blocks)
                for i, (kt, c, w) in enumerate(blocks):
                    diag = kt >= 4 * qs
                    st = ps_st.tile([128, QSL], f32, tag="st", name=f"st_{h}_{qs}_{kt}")
                    nc.tensor.matmul(
                        st[:, 0:w],
                        lhsT=qkt_sb[:, k_ci, kt * KTL:(kt + 1) * KTL],
                        rhs=qkt_sb[:, q_ci, qs * QSL + c:(qs + 1) * QSL],
                        start=True,
                        stop=True,
                    )
                    e_t = epool.tile([128, 2, QSL], bf, tag="etile", name=f"e_{h}_{qs}_{kt}")
                    nc.scalar.activation(
                        out=e_t[:, 0, 0:w],
                        in_=st[:, 0:w],
                        func=AFT.Exp,
                        bias=alibi_sb[:, h * N_KT + kt:h * N_KT + kt + 1],
                        scale=1.0,
                    )
                    if diag:
                        nc.vector.tensor_mul(
                            out=e_t[:, 0, 0:128], in0=e_t[:, 0, 0:128], in1=tri_sb
                        )
                    first, last = (i == 0), (i == nb - 1)
                    nc.tensor.matmul(
                        ctx_ps[:, c:QSL],
                        lhsT=v_sb[:, kt, h * 128:(h + 1) * 128],
                        rhs=e_t[:, 0, 0:w],
                        start=first,
                        stop=last,
                        skip_group_check=True,
                    )
                    nc.tensor.matmul(
                        sums_ps[:, c:QSL],
                        lhsT=ones_bf,
                        rhs=e_t[:, 0, 0:w],
                        start=first,
                        stop=last,
                        skip_group_check=True,
                    )
                recipb = rpool.tile([128, QSL], f32, tag="recipb", name=f"recipb_{h}_{qs}")
                nc.vector.reciprocal_approx_fast(out=recipb, in_=sums_ps)
                nc.vector.tensor_mul(
                    out=ctx_sb[:, h, qs * QSL:(qs + 1) * QSL],
                    in0=ctx_ps,
                    in1=recipb,
                )

            attention_qs = attention_qs_fp8 if ATTN_FP8 else attention_qs_bf16

            def dense_so(so):
                """dense partial for token tile so: [128 s, 2048 h']."""
                for hs in range(4):
                    psd = ps_qk.tile([128, QSL], f32, tag="qk", name=f"d_{so}_{hs}")
                    for ct in range(HPC):
                        nc.tensor.matmul(
                            psd,
                            lhsT=ctx_sb[:, ct, so * 128:(so + 1) * 128],
                            rhs=wd_sb[:, ct, hs * QSL:(hs + 1) * QSL],
                            start=(ct == 0),
                            stop=(ct == HPC - 1),
                        )
                    ot = outstage.tile([128, QSL], bf, tag="ostage", name=f"o_{so}_{hs}")
                    if hs % 2 == 0:
                        nc.vector.tensor_copy(out=ot, in_=psd)
                    else:
                        nc.scalar.copy(out=ot, in_=psd)
                    nc.sync.dma_start(
                        out=out_r[:, so, hs * QSL:(hs + 1) * QSL], in_=ot
                    )

            # ------------- per-head pipeline -------------
            for h in range(HPC):
                if h < HPC - 1:
                    for ci in (2 * h + 2, 2 * h + 3):
                        wt = wstream.tile([128, N_HT, 128], bf, tag="wstream", name=f"wqk_{ci}")
                        nc.sync.dma_start(out=wt, in_=wqk_d[:, ci])
                        wtiles[ci] = wt
                for qs in range(N_QS):
                    attention_qs(h, qs)
                    if h < HPC - 1:
                        qk_unit(2 * h + 2, qs)
                        qk_unit(2 * h + 3, qs)
                    else:
                        for so in range(4 * qs, 4 * qs + 4):
                            dense_so(so)

    nc.compile()
    return nc


def _prepare_core_inputs(inputs):
    hs = np.asarray(inputs["hidden_states"], np.float32)
    alibi = np.asarray(inputs["alibi"], np.float32).reshape(B, NH, S)
    W_qkv = np.asarray(inputs["W_qkv"], np.float32).reshape(H, NH, 3, HD)
    b_qkv = np.asarray(inputs["b_qkv"], np.float32).reshape(NH, 3, HD)
    W_dense = np.asarray(inputs["W_dense"], np.float32)

    mask = np.asarray(inputs["attention_mask"], bool)
    causal = np.triu(np.ones((S, S), bool), k=1)
    assert all(np.array_equal(mask[b, 0], causal) for b in range(mask.shape[0])), \
        "kernel hardcodes the causal mask"

    mdt = f8 if ATTN_FP8 else bf16
    # partition-major: [128, 16, S] with xt[p, ho, s] = hs[s, ho*128+p]
    xt = [
        np.ascontiguousarray(
            hs[b].T.reshape(N_HT, 128, S).transpose(1, 0, 2)
        ).astype(bf16)
        for b in range(B)
    ]
    tri = np.tril(np.ones((128, 128), np.float32)).T.astype(mdt)  # allow[k,q]=q>=k
    ztri = np.concatenate(
        [np.zeros((128, 128), np.float32), np.tril(np.ones((128, 128), np.float32)).T],
        axis=1,
    ).astype(mdt)

    in_maps = []
    for c in range(NCORES):
        b, g = divmod(c, TP)
        heads = range(HPC * g, HPC * g + HPC)
        wqk = np.empty((H, NCI * 128), np.float32)
        bqk = np.empty((NCI, 128), np.float32)
        wv = np.empty((H, HPC * 128), np.float32)
        wd = np.empty((HPC * 128, H), np.float32)
        al = np.empty((128, HPC * N_KT), np.float32)
        for i, hh in enumerate(heads):
            wqk[:, (2 * i) * 128:(2 * i + 1) * 128] = W_qkv[:, hh, 0, :] * INV_NORM
            wqk[:, (2 * i + 1) * 128:(2 * i + 2) * 128] = W_qkv[:, hh, 1, :]
            bqk[2 * i] = b_qkv[hh, 0, :] * INV_NORM
            bqk[2 * i + 1] = b_qkv[hh, 1, :]
            wv[:, i * 128:(i + 1) * 128] = W_qkv[:, hh, 2, :]
            wd[i * 128:(i + 1) * 128, :] = W_dense[hh * HD:(hh + 1) * HD, :]
            al[:, i * N_KT:(i + 1) * N_KT] = (
                alibi[b, hh].reshape(N_KT, 128).T - EXP_SHIFT
            )
        wqk_p = np.ascontiguousarray(
            wqk.reshape(N_HT, 128, NCI, 128).transpose(1, 2, 0, 3)
        )  # [128, NCI, 16, 128]
        wv_p = np.ascontiguousarray(
            wv.reshape(N_HT, 128, HPC * 128).transpose(1, 0, 2)
        )  # [128, 16, 512]
        in_maps.append({
            "xt": xt[b],
            "wqk": wqk_p.astype(bf16),
            "wv": wv_p.astype(bf16),
            "wd": wd.astype(bf16),
            "bqk": np.ascontiguousarray(bqk.T),
            "alibi": al,
            "tri": tri,
            "ztri": ztri,
        })
    return in_maps


def _run(inputs, trace=False, trace_cores=None):
    from concourse.bass_utils import run_bass_kernel_spmd

    in_maps = _prepare_core_inputs(inputs)
    key = ("v31", ATTN_FP8)
    if key not in _program_cache:
        _program_cache[key] = _build_program()
    nc = _program_cache[key]
    res = run_bass_kernel_spmd(
        nc,
        in_maps,
        core_ids=list(range(NCORES)),
        trace=trace,
        trace_cores=trace_cores,
    )

    residual = np.asarray(inputs["residual"], np.float32)
    b_dense = np.asarray(inputs["b_dense"], np.float32)
    b_qkv = np.asarray(inputs["b_qkv"], np.float32).reshape(NH, 3, HD)
    W_dense = np.asarray(inputs["W_dense"], np.float32)
    bv_full = b_qkv[:, 2, :].reshape(H)
    b_eff = b_dense + bv_full @ W_dense  # folded V bias

    out = np.empty((B, S, H), np.float32)
    for b in range(B):
        acc = res.results[b * TP + 0]["out"].astype(np.float32)
        for g in range(1, TP):
            acc += res.results[b * TP + g]["out"].astype(np.float32)
        out[b] = acc + b_eff[None, :] + residual[b]
    return out, res


def kernel(**inputs) -> np.ndarray:
    out, _ = _run(inputs, trace=False)
    return out
